# revision 36
# baseline (speedup 1.0000x reference)
"""Trainium2 Bass kernel for block-causal masked multi-head self-attention.

Module: y = proj(softmax(mask(QK^T/sqrt(D))) V) for B=4, T=2048, C=512, H=8,
with a frame-block-causal mask (frame = t//4) and a per-key validity mask.

Sharding: 8 cores = 4 batches x 2 head-groups (4 heads each). Each core
computes QKV projections for its heads, flash-style attention, and a partial
output projection over its 256 channels; the host sums the two partial
projections per batch and adds the projection bias.

Device-side tricks (all validated against numpy):
  - Scores are computed transposed (keys on partitions, queries on free dim)
    so no transposes are needed anywhere.
  - The frame-causal mask inside a diagonal 128-block is folded into the
    QK^T matmul via 32 extra contraction rows: one-hot(frame(k)) on the K
    side against -640*[i > frame(q)] on the Q side.
  - Per-key validity masking is done by zeroing rows of V' = [V | 1]; the
    appended ones-column makes the PV matmul produce softmax denominators
    for free (no max-subtraction needed: scores are O(1)).
  - exp() has no bias/scale: the 1/sqrt(D) scale is folded into Wq on host.
"""

import math

import numpy as np

B, T, C = 4, 2048, 512
H, NOBJ, D = 8, 4, 64
NCORES = 8
HPC = 4  # heads per core
NKB = 16  # key blocks of 128
QCN = 4  # query chunks of 512
GRP = 2  # full key-blocks per PSUM group ([128, 1024] = 2 banks)

_CACHE = {}


def _apply_tile_patch(tile_mod, mybir):
    """walrus in this container rejects >1 semaphore wait per instruction;
    spread the TileContext tail-drain waits over sync NOPs (the rest of the
    module is handled by _split_multi_waits after lowering)."""
    import bass_rust

    if getattr(tile_mod.TileContext, "_drain_patched", False):
        return

    def _drain_and_barrier(self, tick_clock, wait_clock):
        nc = self.nc
        drain_inst = nc.sync.drain()
        wait_clock.add_sem_waits(
            drain_inst.ins, bass_rust.ScopedClock({None: tick_clock.global_clock})
        )
        waits = list(drain_inst.ins.sync_info.on_wait or [])
        if len(waits) > 1:
            drain_inst.ins.sync_info.on_wait = []
            for w in waits:
                nop = nc.sync.nop(nofuse=True)
                nop.ins.sync_info = mybir.SyncInfo(on_wait=[w], on_update=[])
            nc.sync.drain()
        nc.all_engine_barrier()
        assert self.sems is not None
        popped = nc._tile_sem_poison_stack.pop()
        assert popped is self._sem_poison
        nc.clear_and_free_semaphores(list(self.sems.allocated().values()))
        nc.all_engine_barrier()

    tile_mod.TileContext._drain_and_barrier = _drain_and_barrier
    tile_mod.TileContext._drain_patched = True


def _split_multi_waits(nc, mybir):
    """Post-pass: for every instruction carrying more than one semaphore
    wait, hoist the extra waits onto same-engine NOPs inserted immediately
    before it (engines execute serially, so blocking at the NOP is
    equivalent)."""
    nonce = 0
    for fn in nc.m.functions:
        for blk in fn.blocks:
            insts = list(blk.instructions)
            out = []
            changed = False
            for ins in insts:
                si = ins.sync_info
                waits = list(si.on_wait) if si and si.on_wait else []
                if len(waits) > 1:
                    changed = True
                    for w in waits[:-1]:
                        nop = mybir.InstNoOp(
                            name=f"I-waitsplit-{nonce}", ins=[], outs=[]
                        )
                        nonce += 1
                        nop.engine = ins.engine
                        nop.sync_info = mybir.SyncInfo(on_wait=[w], on_update=[])
                        nc.register_instruction(nop, overwrite=True)
                        out.append(nop)
                    ins.sync_info.on_wait = waits[-1:]
                out.append(ins)
            if changed:
                blk.instructions = out


def _build_program(mm_dtype_name="float32r"):
    import concourse.bass as bass
    import concourse.mybir as mybir
    import concourse.tile as tile

    _apply_tile_patch(tile, mybir)

    f32 = mybir.dt.float32
    bf16 = mybir.dt.bfloat16
    mmdt = getattr(mybir.dt, mm_dtype_name)
    EXP = mybir.ActivationFunctionType.Exp

    nc = bass.Bass(trn_type="TRN2")

    xt = nc.dram_tensor("xt", [C, T], mmdt, kind="ExternalInput")
    wq = nc.dram_tensor("wq", [C, 256], mmdt, kind="ExternalInput")
    wk = nc.dram_tensor("wk", [C, 256], mmdt, kind="ExternalInput")
    wv = nc.dram_tensor("wv", [C, 260], mmdt, kind="ExternalInput")
    bqk = nc.dram_tensor("bqk", [4, 128], f32, kind="ExternalInput")
    bv = nc.dram_tensor("bv", [1, 260], mmdt, kind="ExternalInput")
    wp = nc.dram_tensor("wp", [256, 512], mmdt, kind="ExternalInput")
    vm = nc.dram_tensor("vm", [NKB, 128], f32, kind="ExternalInput")
    aq = nc.dram_tensor("aq", [32, T], mmdt, kind="ExternalInput")
    ak = nc.dram_tensor("ak", [32, T], mmdt, kind="ExternalInput")
    konst = nc.dram_tensor("konst", [2, 512], mmdt, kind="ExternalInput")
    out = nc.dram_tensor("out", [T, C], f32, kind="ExternalOutput")

    def mm(o, lhsT, rhs, start, stop):
        nc.tensor.matmul(o, lhsT, rhs, start=start, stop=stop)

    with nc.allow_low_precision(
        reason="fp32r matmul inputs; PSUM accumulation stays fp32"
    ), tile.TileContext(nc) as tc:
        with tc.tile_pool(name="const", bufs=1) as cp:
            wq_s = cp.tile([128, 4 * 256], mmdt)
            wk_s = cp.tile([128, 4 * 256], mmdt)
            wv_s = cp.tile([128, 4 * 260], mmdt)
            wp_s = cp.tile([128, 2 * 512], mmdt)
            bqk_s = cp.tile([128, 4], f32)
            bv_s = cp.tile([1, 260], mmdt)
            vm_s = cp.tile([128, NKB], f32)
            ones_s = cp.tile([1, 512], mmdt)
            zer_s = cp.tile([1, 65], mmdt)
            qtd = [cp.tile([128, T], mmdt, tag=f"qtd{h}", name=f"qtd{h}") for h in range(HPC)]
            ktd = [cp.tile([128, T], mmdt, tag=f"ktd{h}", name=f"ktd{h}") for h in range(HPC)]
            v4 = cp.tile([128, NKB * 260], mmdt)

            for kc in range(4):
                nc.scalar.dma_start(wq_s[:, kc * 256 : (kc + 1) * 256], wq[kc * 128 : (kc + 1) * 128, :])
                nc.scalar.dma_start(wk_s[:, kc * 256 : (kc + 1) * 256], wk[kc * 128 : (kc + 1) * 128, :])
            nc.gpsimd.dma_start(bqk_s[:], bqk[:].rearrange("n p -> p n"))
            nc.gpsimd.dma_start(vm_s[:], vm[:].rearrange("n p -> p n"))
            nc.scalar.dma_start(ones_s[:], konst[0:1, :])
            nc.scalar.dma_start(zer_s[:], konst[1:2, 0:65])

            # ---- Phase A: QKV projections ----
            with tc.tile_pool(name="xtp", bufs=1) as xp:
                xts = xp.tile([128, 4 * T], mmdt)
                for qb in range(4):
                    eng = nc.sync if qb % 2 == 0 else nc.scalar
                    for kc in range(4):
                        eng.dma_start(
                            xts[:, kc * T + qb * 512 : kc * T + (qb + 1) * 512],
                            xt[kc * 128 : (kc + 1) * 128, qb * 512 : (qb + 1) * 512],
                        )
                for kc in range(4):
                    nc.sync.dma_start(wv_s[:, kc * 260 : (kc + 1) * 260], wv[kc * 128 : (kc + 1) * 128, :])
                for rc in range(2):
                    nc.sync.dma_start(wp_s[:, rc * 512 : (rc + 1) * 512], wp[rc * 128 : (rc + 1) * 128, :])
                nc.sync.dma_start(bv_s[:], bv[:])
                for h in range(HPC):
                    nc.scalar.dma_start(qtd[h][64:96, :], aq[:])
                    nc.scalar.dma_start(ktd[h][64:96, :], ak[:])

                with tc.tile_pool(name="pjq", bufs=4, space="PSUM") as pjq, tc.tile_pool(
                    name="pjv", bufs=2, space="PSUM"
                ) as pjv:
                    for qb in range(4):
                        for ti, (w_s, bcol0, dst) in enumerate(
                            [(wq_s, 0, qtd), (wk_s, 2, ktd)]
                        ):
                            for rc in range(2):
                                ps = pjq.tile([128, 512], f32, tag="pj")
                                for kc in range(4):
                                    mm(
                                        ps[:],
                                        w_s[:, kc * 256 + rc * 128 : kc * 256 + rc * 128 + 128],
                                        xts[:, kc * T + qb * 512 : kc * T + qb * 512 + 512],
                                        kc == 0,
                                        kc == 3,
                                    )
                                for hh in range(2):
                                    h = 2 * rc + hh
                                    nc.vector.tensor_scalar_add(
                                        dst[h][0:64, qb * 512 : (qb + 1) * 512],
                                        ps[hh * 64 : (hh + 1) * 64, :],
                                        bqk_s[hh * 64 : (hh + 1) * 64, bcol0 + rc : bcol0 + rc + 1],
                                    )
                    # V (keys on partitions), bias + ones column via K=1 init matmul
                    for kb in range(NKB):
                        ps = pjv.tile([128, 260], f32, tag="pv")
                        mm(ps[:], ones_s[:, 0:128], bv_s[:], True, False)
                        for kc in range(4):
                            mm(
                                ps[:],
                                xts[:, kc * T + kb * 128 : kc * T + kb * 128 + 128],
                                wv_s[:, kc * 260 : (kc + 1) * 260],
                                False,
                                kc == 3,
                            )
                        nc.scalar.activation(
                            v4[:, kb * 260 : (kb + 1) * 260], ps[:],
                            mybir.ActivationFunctionType.Copy,
                            bias=0.0, scale=vm_s[:, kb : kb + 1],
                        )

            # ---- Phase B: attention + output projection ----
            with tc.tile_pool(name="attp", bufs=2, space="PSUM") as attp, tc.tile_pool(
                name="ytp", bufs=3, space="PSUM"
            ) as ytp, tc.tile_pool(
                name="rtp", bufs=1, space="PSUM"
            ) as rtp, tc.tile_pool(
                name="psb", bufs=3
            ) as psb, tc.tile_pool(name="yhp", bufs=2) as yhp, tc.tile_pool(
                name="osp", bufs=2
            ) as osp, tc.tile_pool(name="rrp", bufs=2) as rrp:
                # diag-band PSUM layout: d0 f0 d1 f1 d2 f2 d3
                dg_off = [0, 512, 640, 1024, 1152]  # recomputed below
                for qc in range(QCN):
                    yh = [yhp.tile([128, 512], mmdt, tag=f"yh{c}", name=f"yh{c}") for c in range(2)]
                    for h in range(HPC):
                        yt = ytp.tile([128, 512], f32, tag="yt")
                        if qc == 0:
                            mm(yt[0:65, :], zer_s[:], ones_s[:, 0:512], True, False)
                        first_pv = qc != 0
                        qsl = slice(qc * 512, (qc + 1) * 512)
                        full_kbs = list(range(4 * qc))
                        groups = [full_kbs[i : i + GRP] for i in range(0, len(full_kbs), GRP)]
                        for kbs in groups:
                            n = len(kbs)
                            at_p = attp.tile([128, GRP * 512], f32, tag="att")
                            for i, kb in enumerate(kbs):
                                mm(
                                    at_p[:, i * 512 : (i + 1) * 512],
                                    ktd[h][0:64, kb * 128 : (kb + 1) * 128],
                                    qtd[h][0:64, qsl],
                                    True,
                                    True,
                                )
                            p_t = psb.tile([128, GRP * 512], mmdt, tag="p")
                            nc.scalar.activation(p_t[:, 0 : n * 512], at_p[:, 0 : n * 512], EXP)
                            for i, kb in enumerate(kbs):
                                mm(
                                    yt[0:65, :],
                                    v4[:, kb * 260 + h * 65 : kb * 260 + h * 65 + 65],
                                    p_t[:, i * 512 : (i + 1) * 512],
                                    first_pv,
                                    False,
                                )
                                first_pv = False
                        # diagonal band: kb = 4qc+j ; d_j = own q-tile (with
                        # one-hot causal rows), f_j = remaining fully-allowed
                        # tiles. Split into two <=1024-wide PSUM groups.
                        for half, js in enumerate(((0, 1), (2, 3))):
                            at_p = attp.tile([128, GRP * 512], f32, tag="att", name=f"atd{half}")
                            segs = []  # (kb, ycol, psum_off, width) for merged PV
                            off = 0
                            for j in js:
                                kb = 4 * qc + j
                                # d_j: own q-tile with one-hot causal rows. Pad
                                # to N=256 (fp32r runs 4x slower below 256);
                                # padded columns hold garbage that the next
                                # segment (or unread tail) overwrites/ignores.
                                dn = 128
                                if qc * 512 + j * 128 + 256 <= T and off + 256 <= GRP * 512:
                                    dn = 256
                                mm(
                                    at_p[:, off : off + dn],
                                    ktd[h][0:96, kb * 128 : (kb + 1) * 128],
                                    qtd[h][0:96, qc * 512 + j * 128 : qc * 512 + j * 128 + dn],
                                    True,
                                    True,
                                )
                                fw = 512 - (j + 1) * 128
                                segs.append((kb, j * 128, off, 128 + fw))
                                off += 128
                                if fw > 0:
                                    fn = fw
                                    if (
                                        fw < 256
                                        and off + 256 <= GRP * 512
                                        and qc * 512 + (j + 1) * 128 + 256 <= T
                                    ):
                                        fn = 256
                                    mm(
                                        at_p[:, off : off + fn],
                                        ktd[h][0:64, kb * 128 : (kb + 1) * 128],
                                        qtd[h][0:64, qc * 512 + (j + 1) * 128 : qc * 512 + (j + 1) * 128 + fn],
                                        True,
                                        True,
                                    )
                                    off += fw
                            p_t = psb.tile([128, GRP * 512], mmdt, tag="p")
                            nc.scalar.activation(p_t[:, 0:off], at_p[:, 0:off], EXP)
                            for kb, ycol, poff, w in segs:
                                mm(
                                    yt[0:65, ycol : ycol + w],
                                    v4[:, kb * 260 + h * 65 : kb * 260 + h * 65 + 65],
                                    p_t[:, poff : poff + w],
                                    first_pv,
                                    kb == 4 * qc + 3,
                                )
                                first_pv = False
                        # normalize: yh_h = yt[0:64] * (1 / yt[64]) broadcast
                        rr = rrp.tile([1, 512], mmdt, tag="rr")
                        nc.vector.reciprocal(rr[0:1, :], yt[64:65, :])
                        rt = rtp.tile([64, 512], f32, tag="rt")
                        mm(rt[:], ones_s[:, 0:64], rr[:], True, True)
                        rt_sb = rrp.tile([64, 512], f32, tag="rtsb")
                        nc.vector.tensor_copy(rt_sb[:], rt[0:64, :])
                        nc.vector.tensor_mul(
                            yh[h // 2][(h % 2) * 64 : (h % 2) * 64 + 64, :],
                            yt[0:64, :],
                            rt_sb[:],
                        )
                    # output projection for this q-chunk (4 q-tiles)
                    for j in range(4):
                        qt = 4 * qc + j
                        po = ytp.tile([128, 512], f32, tag="yt", name=f"po{qc}{j}")
                        for cch in range(2):
                            mm(
                                po[:, 0:512],
                                yh[cch][:, j * 128 : (j + 1) * 128],
                                wp_s[:, cch * 512 : (cch + 1) * 512],
                                cch == 0,
                                cch == 1,
                            )
                        os_t = osp.tile([128, 512], f32, tag="os")
                        nc.vector.tensor_copy(os_t[:], po[:, 0:512])
                        nc.sync.dma_start(out[qt * 128 : (qt + 1) * 128, :], os_t[:])
    _split_multi_waits(nc, mybir)
    return nc


def _host_inputs(x, mask, Wq, bq, Wk, bk, Wv, bv, Wp, bp):
    """Build the per-core input maps."""
    scale = 1.0 / math.sqrt(D)
    # one-hot / penalty patterns for the in-matmul diagonal causal mask
    u = np.arange(T) % 128
    fr = u // NOBJ  # frame within 128-tile, 0..31
    i_idx = np.arange(32)[:, None]
    ak_host = (fr[None, :] == i_idx).astype(np.float32)
    konst_host = np.zeros((2, 512), np.float32)
    konst_host[0, :] = 1.0
    aq_host = np.where(i_idx > fr[None, :], np.float32(-640.0), np.float32(0.0))

    in_maps = []
    for c in range(NCORES):
        b, g = divmod(c, 2)
        ch = slice(g * 256, (g + 1) * 256)
        wq_h = np.ascontiguousarray((Wq[ch, :] * scale).T)  # [512, 256]
        wk_h = np.ascontiguousarray(Wk[ch, :].T)
        wv_flat = Wv[ch, :].T  # [512, 256]
        wv_h = np.zeros((C, 260), np.float32)
        bv_h = np.zeros((1, 260), np.float32)
        bvc = bv[ch]
        for h in range(HPC):
            wv_h[:, h * 65 : h * 65 + 64] = wv_flat[:, h * 64 : (h + 1) * 64]
            bv_h[0, h * 65 : h * 65 + 64] = bvc[h * 64 : (h + 1) * 64]
            bv_h[0, h * 65 + 64] = 1.0
        bq_h = bq[ch] * scale
        bk_h = bk[ch]
        bqk_h = np.stack([bq_h[:128], bq_h[128:], bk_h[:128], bk_h[128:]])
        in_maps.append(
            {
                "xt": np.ascontiguousarray(x[b].T).astype(np.float32),
                "wq": wq_h.astype(np.float32),
                "wk": wk_h.astype(np.float32),
                "wv": wv_h,
                "bqk": bqk_h.astype(np.float32),
                "bv": bv_h,
                "wp": np.ascontiguousarray(Wp[:, ch].T).astype(np.float32),
                "vm": mask[b].astype(np.float32).reshape(NKB, 128),
                "aq": aq_host,
                "ak": ak_host,
                "konst": konst_host,
            }
        )
    return in_maps


def kernel(x, mask, Wq, bq, Wk, bk, Wv, bv, Wp, bp):
    from concourse.bass_utils import run_bass_kernel_spmd

    if "nc" not in _CACHE:
        _CACHE["nc"] = _build_program()
    nc = _CACHE["nc"]

    in_maps = _host_inputs(
        np.asarray(x), np.asarray(mask),
        np.asarray(Wq), np.asarray(bq), np.asarray(Wk), np.asarray(bk),
        np.asarray(Wv), np.asarray(bv), np.asarray(Wp), np.asarray(bp),
    )
    res = run_bass_kernel_spmd(nc, in_maps, core_ids=list(range(NCORES)))
    outs = [res.results[c]["out"] for c in range(NCORES)]
    y = np.empty((B, T, C), np.float32)
    for b in range(B):
        y[b] = outs[2 * b] + outs[2 * b + 1] + np.asarray(bp)[None, :]
    return y
